# revision 22
# baseline (speedup 1.0000x reference)
"""Multi-head attention (2 batches x 4 heads, n=4096, dh=128) on 8 trn2 cores.

Sharding: one (batch, head) pair per NeuronCore (batch*heads = 8 = n_cores).
Per core, the full attention for its head runs on-chip:
  qkv projection (fp32r matmuls) -> S^T = K_d^T Q_d chunks -> exp on ACT
  (fp16, scale=1/sqrt(dh), bias=-2 folded into the activation) -> P^T V
  accumulated in PSUM -> normalize by the softmax denominator computed via a
  fp16 add-tree (DVE) + ones-matmul partition reduction.

Host side only shards/assembles: x[b] per core, W/b sliced per head (heads are
interleaved in the qkv projection: feature d*HEADS + h), output per-head
columns concatenated.
"""

import numpy as np
from contextlib import ExitStack

import concourse.bass as bass
import concourse.mybir as mybir
import concourse.tile as tile
from concourse.bass_utils import run_bass_kernel_spmd
from concourse.masks import make_identity
from bass_rust import ScopedClock

F32 = mybir.dt.float32
F32R = mybir.dt.float32r
F16 = mybir.dt.float16
AF = mybir.ActivationFunctionType

B = 2
HEADS = 4
N = 4096
DIM = 512
DH = 128
NCORES = 8

SCALE = DH ** -0.5        # folded into the exp activation
EXP_BIAS = -2.0           # exp(s*SCALE - 2): keeps fp16 sums < ~5e3 (max |s*SCALE| ~ 2.1)

NG = 8                    # query groups of 512
QG = 512                  # queries per group
KC = 32                   # key chunks of 128
# exp spans: chunks of S^T banks consumed per ACT instruction
def spans():
    w = CFG["span"]
    out, c = [], 0
    while c < KC:
        out.append((c, min(w, KC - c)))
        c += w
    return out


MAXW = 1  # max sync waits this walrus build accepts per instruction

CFG = {"xin": 6, "xtp": 2, "vtmp": 2, "ps_tr": 4, "ps_mm": 2, "ps_v": 2, "span": 3, "st_bufs": 2}


class _TC(tile.TileContext):
    """TileContext with a post-pass that splits instructions' sem waits
    across preceding same-engine NOPs: this container's walrus rejects any
    instruction carrying more than MAXW sync waits (CoreV3 setupSyncWait:
    "Too many sync wait commands")."""

    def _drain_and_barrier(self, tick_clock, wait_clock):
        nc = self.nc
        drain_inst = nc.sync.drain()
        wait_clock.add_sem_waits(
            drain_inst.ins, ScopedClock({None: tick_clock.global_clock})
        )
        nc.all_engine_barrier()
        assert self.sems is not None
        popped = nc._tile_sem_poison_stack.pop()
        assert popped is self._sem_poison
        nc.clear_and_free_semaphores(list(self.sems.allocated().values()))
        nc.all_engine_barrier()
        self._split_excess_waits()

    def _split_excess_waits(self):
        nc = self.nc
        cur_insts = nn_bb_insts(nc)
        for bb in nc.m.functions[0].blocks:
            insts = bb.instructions
            pos = 0
            while pos < len(insts):
                inst = insts[pos]
                si = inst.sync_info
                waits = list(si.on_wait) if si and si.on_wait else []
                if len(waits) <= MAXW:
                    pos += 1
                    continue
                si.on_wait = waits[-MAXW:]
                rest = waits[:-MAXW]
                eng = nc.engines[inst.engine]
                for i in range(0, len(rest), MAXW):
                    chunk = rest[i : i + MAXW]
                    nop = eng.nop()
                    # relocate the freshly appended nop from cur_bb's tail
                    # to just before the offending instruction
                    popped = cur_insts.pop()
                    assert popped.name == nop.ins.name
                    nsi = nop.ins.sync_info
                    if nsi is None:
                        nop.ins.sync_info = mybir.SyncInfo(
                            on_wait=chunk, on_update=[]
                        )
                    else:
                        nsi.on_wait = chunk
                    insts.insert(pos, nop.ins)
                    pos += 1
                pos += 1


def nn_bb_insts(nc):
    bb = nc.cur_bb
    assert bb is not None
    return bb.bb.instructions


def build(repeat=1, skip=(), loop_reps=None):
    nc = bass.Bass()
    xb = nc.dram_tensor("xb", [N, DIM], F32R, kind="ExternalInput")
    # per-head W^T, columns [q | k | v], each [DIM, DH]
    wt = nc.dram_tensor("wt", [DIM, 3 * DH], F32R, kind="ExternalInput")
    bqkv = nc.dram_tensor("bqkv", [3, DH], F32, kind="ExternalInput")
    y = nc.dram_tensor("y", [N, DH], F32, kind="ExternalOutput")

    with ExitStack() as ctx:
        tc = ctx.enter_context(_TC(nc))

        singles = ctx.enter_context(tc.tile_pool(name="singles", bufs=1))

        identsrc = singles.tile([128, 128], F32)
        make_identity(nc, identsrc)
        ident = singles.tile([128, 128], F32R)
        nc.vector.tensor_copy(out=ident, in_=identsrc)
        ones16 = singles.tile([128, 1], F16)
        nc.vector.memset(ones16, 1.0)
        expb = singles.tile([128, 1], F32)
        nc.vector.memset(expb, EXP_BIAS)

        # weights [dm-within-chunk, dm-chunk, 3*dh] and biases [dh, 3]
        wt_sb = singles.tile([128, 4, 3 * DH], F32R)
        nc.sync.dma_start(out=wt_sb, in_=wt[:, :].rearrange("(c p) o -> p c o", p=128))
        b_sb = singles.tile([128, 3], F32)
        nc.sync.dma_start(out=b_sb, in_=bqkv[:, :].rearrange("t d -> d t"))

        # resident activations
        qd = singles.tile([128, N], F32R)            # Q^T  [dh, n]
        kd = singles.tile([128, N], F32R)            # K^T  [dh, n]
        vsb = singles.tile([128, KC, DH], F16)      # V    [n-in-chunk, chunk, dh]

        if loop_reps is None:
            for _rep in range(repeat):
                _body(nc, tc, ident, ones16, expb, wt_sb, b_sb, qd, kd, vsb, xb, y, skip)
        else:
            with tc.For_i(0, loop_reps, 1):
                _body(nc, tc, ident, ones16, expb, wt_sb, b_sb, qd, kd, vsb, xb, y, skip)

    return nc


def _body(nc, tc, ident, ones16, expb, wt_sb, b_sb, qd, kd, vsb, xb, y, skip=()):
    if True:
        # ---------------- phase 1: qkv projection ----------------
        ph1 = ExitStack()
        xin = ph1.enter_context(tc.tile_pool(name="xin", bufs=CFG["xin"]))
        xtp = ph1.enter_context(tc.tile_pool(name="xtp", bufs=CFG["xtp"]))
        vtmp = ph1.enter_context(tc.tile_pool(name="vtmp", bufs=CFG["vtmp"]))
        ps_tr = ph1.enter_context(tc.tile_pool(name="ps_tr", bufs=CFG["ps_tr"], space="PSUM"))
        ps_mm = ph1.enter_context(tc.tile_pool(name="ps_mm", bufs=CFG["ps_mm"], space="PSUM"))
        ps_v = ph1.enter_context(tc.tile_pool(name="ps_v", bufs=CFG["ps_v"], space="PSUM"))

        for nch in range(8) if "ph1" not in skip else []:  # 512-token chunks
            xt_all = xtp.tile([128, 4, 512], F32R)    # x^T [dm-part, dm-chunk, n]
            for j in range(4):
                xtile = xin.tile([128, DIM], F32R)
                nc.sync.dma_start(
                    out=xtile, in_=xb[nch * 512 + j * 128 : nch * 512 + (j + 1) * 128, :]
                )
                tps = ps_tr.tile([128, 512], F32)
                for d in range(4):
                    nc.tensor.transpose(
                        tps[:, d * 128 : (d + 1) * 128].bitcast(F32R),
                        xtile[:, d * 128 : (d + 1) * 128],
                        ident,
                    )
                # scatter the 4 transposed dm-chunks into x^T (alternate
                # engines so neither ACT nor DVE serializes phase 1)
                if j % 2 == 0:
                    nc.scalar.copy(
                        xt_all[:, :, j * 128 : (j + 1) * 128],
                        tps.rearrange("p (d n) -> p d n", d=4),
                    )
                else:
                    nc.vector.tensor_copy(
                        out=xt_all[:, :, j * 128 : (j + 1) * 128],
                        in_=tps.rearrange("p (d n) -> p d n", d=4),
                    )
            for m in range(3):                       # q, k, v
                pm = ps_mm.tile([128, 512], F32)
                for d in range(4):
                    nc.tensor.matmul(
                        pm,
                        lhsT=wt_sb[:, d, m * DH : (m + 1) * DH],
                        rhs=xt_all[:, d, :],
                        start=(d == 0),
                        stop=(d == 3),
                    )
                if m == 0:
                    nc.vector.tensor_scalar_add(
                        qd[:, nch * 512 : (nch + 1) * 512], pm, b_sb[:, 0:1]
                    )
                elif m == 1:
                    nc.vector.tensor_scalar_add(
                        kd[:, nch * 512 : (nch + 1) * 512], pm, b_sb[:, 1:2]
                    )
                else:
                    vt = vtmp.tile([128, 512], F32R)
                    nc.vector.tensor_scalar_add(vt, pm, b_sb[:, 2:3])
                    for j in range(4):
                        tv = ps_v.tile([128, 128], F32)
                        nc.tensor.transpose(
                            tv.bitcast(F32R),
                            vt[:, j * 128 : (j + 1) * 128],
                            ident,
                        )
                        nc.scalar.copy(vsb[:, nch * 4 + j, :], tv)

        ph1.close()

        # ---------------- phase 2: attention ----------------
        ph2 = ExitStack()
        pt_pool = ph2.enter_context(tc.tile_pool(name="pt", bufs=24))
        acc_pool = ph2.enter_context(tc.tile_pool(name="acc", bufs=10))
        cs_pool = ph2.enter_context(tc.tile_pool(name="cs", bufs=2))
        ot_pool = ph2.enter_context(tc.tile_pool(name="ot", bufs=3))
        ob_pool = ph2.enter_context(tc.tile_pool(name="ob", bufs=3))
        rc_pool = ph2.enter_context(tc.tile_pool(name="rc", bufs=4))
        ps_st = ph2.enter_context(tc.tile_pool(name="ps_st", bufs=CFG["st_bufs"], space="PSUM"))
        ps_pv = ph2.enter_context(tc.tile_pool(name="ps_pv", bufs=1, space="PSUM"))
        ps_sm = ph2.enter_context(tc.tile_pool(name="ps_sm", bufs=1, space="PSUM"))

        for g in range(NG) if "attn" not in skip else []:
            q_sl = slice(g * QG, (g + 1) * QG)
            # S^T chunks -> exp -> P^T (fp16), span by span
            pt_spans = []
            SP = spans()
            for c0, w in SP:
                stp = ps_st.tile([128, 512 * CFG["span"]], F32)
                for j in (range(w) if "st1" not in skip else [0]):
                    kc = c0 + j
                    nc.tensor.matmul(
                        stp[:, j * 512 : (j + 1) * 512],
                        lhsT=kd[:, kc * 128 : (kc + 1) * 128],
                        rhs=qd[:, q_sl],
                        start=True,
                        stop=True,
                    )
                pts = pt_pool.tile([128, 512 * CFG["span"]], F16, tag="pt")
                ew = 1 if "exp3" in skip else w
                nc.scalar.activation(
                    out=pts[:, : ew * 512],
                    in_=stp[:, : ew * 512],
                    func=AF.Exp,
                    scale=SCALE,
                    bias=expb,
                )
                pt_spans.append(pts)

            def ptc(kc):
                s = kc // CFG["span"]
                j = kc - SP[s][0]
                return pt_spans[s][:, j * 512 : (j + 1) * 512]

            # P^T V accumulation: out^T [dh, q]
            pv = ps_pv.tile([128, 512], F32, tag="pv")
            pv_rng = range(KC) if "pv8" not in skip else range(8)
            for kc in pv_rng:
                nc.tensor.matmul(
                    pv,
                    lhsT=vsb[:, kc, :],
                    rhs=ptc(kc),
                    start=(kc == 0),
                    stop=(kc == pv_rng[-1]),
                )

            # denominator: fp16 chunk add-tree (8 lanes x 4 chunks), then
            # 128-partition reduction via ones-matmul
            lanes = []
            if "tree" in skip:
                cs0 = cs_pool.tile([128, 512], F16, name="cs0", tag="cs")
                nc.vector.memset(cs0, 1.0)
            for l in range(8) if "tree" not in skip else []:
                acc = acc_pool.tile([128, 512], F16, tag="acc")
                nc.vector.tensor_add(acc, ptc(4 * l), ptc(4 * l + 1))
                nc.vector.tensor_add(acc, acc, ptc(4 * l + 2))
                nc.vector.tensor_add(acc, acc, ptc(4 * l + 3))
                lanes.append(acc)
            for i in range(4) if "tree" not in skip else []:
                nc.vector.tensor_add(lanes[i], lanes[i], lanes[i + 4])
            for i in range(2) if "tree" not in skip else []:
                nc.vector.tensor_add(lanes[i], lanes[i], lanes[i + 2])
            if "tree" not in skip:
                cs = cs_pool.tile([128, 512], F16)
                nc.vector.tensor_add(cs, lanes[0], lanes[1])
            else:
                cs = cs0

            # out^T -> SBUF, then per-subtile transpose + normalize
            ot = ot_pool.tile([128, 512], F32R)
            nc.scalar.copy(ot, pv)

            dn = ps_sm.tile([128, 4], F32, tag="sm")
            for st in range(4):
                nc.tensor.matmul(
                    dn[:, st : st + 1],
                    lhsT=cs[:, st * 128 : (st + 1) * 128],
                    rhs=ones16,
                    start=True,
                    stop=True,
                )
            rc = rc_pool.tile([128, 4], F32)
            nc.vector.reciprocal(rc, dn)

            tp = ps_sm.tile([128, 512], F32, tag="sm")
            for st in range(4):
                nc.tensor.transpose(
                    tp[:, st * 128 : (st + 1) * 128].bitcast(F32R),
                    ot[:, st * 128 : (st + 1) * 128],
                    ident,
                )
            ob = ob_pool.tile([128, 4, 128], F32)
            for st in range(4):
                nc.vector.tensor_scalar_mul(
                    ob[:, st, :], tp[:, st * 128 : (st + 1) * 128], rc[:, st : st + 1]
                )
            nc.sync.dma_start(
                out=y[q_sl, :].rearrange("(s p) d -> p s d", p=128), in_=ob
            )

        ph2.close()


_NC = None


def kernel(x, W, b):
    global _NC
    if _NC is None:
        _NC = build()

    x = np.asarray(x, dtype=np.float32)
    W = np.asarray(W, dtype=np.float32)
    b = np.asarray(b, dtype=np.float32)

    in_maps = []
    for c in range(NCORES):
        bb, h = divmod(c, HEADS)
        rows = np.arange(DH) * HEADS + h
        wt = np.concatenate(
            [np.ascontiguousarray(W[blk * DIM + rows, :].T) for blk in range(3)],
            axis=1,
        )  # [DIM, 3*DH]
        bs = np.stack([b[blk * DIM + rows] for blk in range(3)], axis=0)  # [3, DH]
        in_maps.append(
            {
                "xb": np.ascontiguousarray(x[bb]),
                "wt": np.ascontiguousarray(wt),
                "bqkv": np.ascontiguousarray(bs),
            }
        )

    res = run_bass_kernel_spmd(_NC, in_maps, core_ids=list(range(NCORES)))

    out = np.empty((B, N, HEADS * DH), dtype=np.float32)
    for c in range(NCORES):
        bb, h = divmod(c, HEADS)
        out[bb, :, h * DH : (h + 1) * DH] = res.results[c]["y"]
    return out


# revision 24
# speedup vs baseline: 1.0340x; 1.0340x over previous
"""Multi-head attention (2 batches x 4 heads, n=4096, dh=128) on 8 trn2 cores.

Sharding: one (batch, head) pair per NeuronCore (batch*heads = 8 = n_cores).
Per core, the full attention for its head runs on-chip:
  qkv projection (fp32r matmuls) -> S^T = K_d^T Q_d chunks -> exp on ACT
  (fp16, scale=1/sqrt(dh), bias=-2 folded into the activation) -> P^T V
  accumulated in PSUM -> normalize by the softmax denominator computed via a
  fp16 add-tree (DVE) + ones-matmul partition reduction.

Host side only shards/assembles: x[b] per core, W/b sliced per head (heads are
interleaved in the qkv projection: feature d*HEADS + h), output per-head
columns concatenated.
"""

import numpy as np
from contextlib import ExitStack

import concourse.bass as bass
import concourse.mybir as mybir
import concourse.tile as tile
from concourse.bass_utils import run_bass_kernel_spmd
from concourse.masks import make_identity
from bass_rust import ScopedClock

F32 = mybir.dt.float32
F32R = mybir.dt.float32r
F16 = mybir.dt.float16
AF = mybir.ActivationFunctionType

B = 2
HEADS = 4
N = 4096
DIM = 512
DH = 128
NCORES = 8

SCALE = DH ** -0.5        # folded into the exp activation
EXP_BIAS = -2.0           # exp(s*SCALE - 2): keeps fp16 sums < ~5e3 (max |s*SCALE| ~ 2.1)

NG = 8                    # query groups of 512
QG = 512                  # queries per group
KC = 32                   # key chunks of 128
# exp spans: chunks of S^T banks consumed per ACT instruction
def spans():
    w = CFG["span"]
    out, c = [], 0
    while c < KC:
        out.append((c, min(w, KC - c)))
        c += w
    return out


MAXW = 1  # max sync waits this walrus build accepts per instruction

CFG = {"xin": 6, "xtp": 2, "vtmp": 2, "ps_tr": 4, "ps_mm": 2, "ps_v": 2, "span": 3, "st_bufs": 2}


class _TC(tile.TileContext):
    """TileContext with a post-pass that splits instructions' sem waits
    across preceding same-engine NOPs: this container's walrus rejects any
    instruction carrying more than MAXW sync waits (CoreV3 setupSyncWait:
    "Too many sync wait commands")."""

    def _drain_and_barrier(self, tick_clock, wait_clock):
        nc = self.nc
        drain_inst = nc.sync.drain()
        wait_clock.add_sem_waits(
            drain_inst.ins, ScopedClock({None: tick_clock.global_clock})
        )
        nc.all_engine_barrier()
        assert self.sems is not None
        popped = nc._tile_sem_poison_stack.pop()
        assert popped is self._sem_poison
        nc.clear_and_free_semaphores(list(self.sems.allocated().values()))
        nc.all_engine_barrier()
        self._split_excess_waits()

    def _split_excess_waits(self):
        nc = self.nc
        cur_insts = nn_bb_insts(nc)
        for bb in nc.m.functions[0].blocks:
            insts = bb.instructions
            pos = 0
            while pos < len(insts):
                inst = insts[pos]
                si = inst.sync_info
                waits = list(si.on_wait) if si and si.on_wait else []
                if len(waits) <= MAXW:
                    pos += 1
                    continue
                si.on_wait = waits[-MAXW:]
                rest = waits[:-MAXW]
                eng = nc.engines[inst.engine]
                for i in range(0, len(rest), MAXW):
                    chunk = rest[i : i + MAXW]
                    nop = eng.nop()
                    # relocate the freshly appended nop from cur_bb's tail
                    # to just before the offending instruction
                    popped = cur_insts.pop()
                    assert popped.name == nop.ins.name
                    nsi = nop.ins.sync_info
                    if nsi is None:
                        nop.ins.sync_info = mybir.SyncInfo(
                            on_wait=chunk, on_update=[]
                        )
                    else:
                        nsi.on_wait = chunk
                    insts.insert(pos, nop.ins)
                    pos += 1
                pos += 1


def nn_bb_insts(nc):
    bb = nc.cur_bb
    assert bb is not None
    return bb.bb.instructions


def build(repeat=1, skip=(), loop_reps=None):
    nc = bass.Bass()
    xb = nc.dram_tensor("xb", [N, DIM], F32R, kind="ExternalInput")
    # per-head W^T, columns [q | k | v], each [DIM, DH]
    wt = nc.dram_tensor("wt", [DIM, 3 * DH], F32R, kind="ExternalInput")
    bqkv = nc.dram_tensor("bqkv", [3, DH], F32, kind="ExternalInput")
    y = nc.dram_tensor("y", [N, DH], F32, kind="ExternalOutput")

    with ExitStack() as ctx:
        tc = ctx.enter_context(_TC(nc))

        singles = ctx.enter_context(tc.tile_pool(name="singles", bufs=1))

        identsrc = singles.tile([128, 128], F32)
        make_identity(nc, identsrc)
        ident = singles.tile([128, 128], F32R)
        nc.vector.tensor_copy(out=ident, in_=identsrc)
        ident16 = singles.tile([128, 128], F16)
        nc.vector.tensor_copy(out=ident16, in_=identsrc)
        ones16 = singles.tile([128, 1], F16)
        nc.vector.memset(ones16, 1.0)
        expb = singles.tile([128, 1], F32)
        nc.vector.memset(expb, EXP_BIAS)

        # weights [dm-within-chunk, dm-chunk, 3*dh] and biases [dh, 3]
        wt_sb32 = singles.tile([128, 4, 3 * DH], F32R)
        nc.sync.dma_start(out=wt_sb32, in_=wt[:, :].rearrange("(c p) o -> p c o", p=128))
        wt_sb = singles.tile([128, 4, 3 * DH], F16)
        nc.vector.tensor_copy(out=wt_sb, in_=wt_sb32)
        b_sb = singles.tile([128, 3], F32)
        nc.sync.dma_start(out=b_sb, in_=bqkv[:, :].rearrange("t d -> d t"))

        # resident activations
        qd = singles.tile([128, N], F32R)            # Q^T  [dh, n]
        kd = singles.tile([128, N], F32R)            # K^T  [dh, n]
        vsb = singles.tile([128, KC, DH], F16)      # V    [n-in-chunk, chunk, dh]

        if loop_reps is None:
            for _rep in range(repeat):
                _body(nc, tc, ident, ident16, ones16, expb, wt_sb, b_sb, qd, kd, vsb, xb, y, skip)
        else:
            with tc.For_i(0, loop_reps, 1):
                _body(nc, tc, ident, ident16, ones16, expb, wt_sb, b_sb, qd, kd, vsb, xb, y, skip)

    return nc


def _body(nc, tc, ident, ident16, ones16, expb, wt_sb, b_sb, qd, kd, vsb, xb, y, skip=()):
    if True:
        # ---------------- phase 1: qkv projection ----------------
        ph1 = ExitStack()
        xin = ph1.enter_context(tc.tile_pool(name="xin", bufs=CFG["xin"]))
        xtp = ph1.enter_context(tc.tile_pool(name="xtp", bufs=CFG["xtp"]))
        vtmp = ph1.enter_context(tc.tile_pool(name="vtmp", bufs=CFG["vtmp"]))
        ps_tr = ph1.enter_context(tc.tile_pool(name="ps_tr", bufs=CFG["ps_tr"], space="PSUM"))
        ps_mm = ph1.enter_context(tc.tile_pool(name="ps_mm", bufs=CFG["ps_mm"], space="PSUM"))
        ps_v = ph1.enter_context(tc.tile_pool(name="ps_v", bufs=CFG["ps_v"], space="PSUM"))

        for nch in range(8) if "ph1" not in skip else []:  # 512-token chunks
            xt_all = xtp.tile([128, 4, 512], F16)     # x^T [dm-part, dm-chunk, n]
            for j in range(4):
                xtile = xin.tile([128, DIM], F32R)
                nc.sync.dma_start(
                    out=xtile, in_=xb[nch * 512 + j * 128 : nch * 512 + (j + 1) * 128, :]
                )
                xtile16 = xin.tile([128, DIM], F16, name="xtile16", tag="xtile16")
                nc.vector.tensor_copy(out=xtile16, in_=xtile)
                tps = ps_tr.tile([128, 512], F16)
                for d in range(4):
                    nc.tensor.transpose(
                        tps[:, d * 128 : (d + 1) * 128],
                        xtile16[:, d * 128 : (d + 1) * 128],
                        ident16,
                    )
                # scatter the 4 transposed dm-chunks into x^T (alternate
                # engines so neither ACT nor DVE serializes phase 1)
                if j % 2 == 0:
                    nc.scalar.copy(
                        xt_all[:, :, j * 128 : (j + 1) * 128],
                        tps.rearrange("p (d n) -> p d n", d=4),
                    )
                else:
                    nc.vector.tensor_copy(
                        out=xt_all[:, :, j * 128 : (j + 1) * 128],
                        in_=tps.rearrange("p (d n) -> p d n", d=4),
                    )
            for m in range(3):                       # q, k, v
                pm = ps_mm.tile([128, 512], F32)
                for d in range(4):
                    nc.tensor.matmul(
                        pm,
                        lhsT=wt_sb[:, d, m * DH : (m + 1) * DH],
                        rhs=xt_all[:, d, :],
                        start=(d == 0),
                        stop=(d == 3),
                    )
                if m == 0:
                    nc.vector.tensor_scalar_add(
                        qd[:, nch * 512 : (nch + 1) * 512], pm, b_sb[:, 0:1]
                    )
                elif m == 1:
                    nc.vector.tensor_scalar_add(
                        kd[:, nch * 512 : (nch + 1) * 512], pm, b_sb[:, 1:2]
                    )
                else:
                    vt = vtmp.tile([128, 512], F32R)
                    nc.vector.tensor_scalar_add(vt, pm, b_sb[:, 2:3])
                    for j in range(4):
                        tv = ps_v.tile([128, 128], F32)
                        nc.tensor.transpose(
                            tv.bitcast(F32R),
                            vt[:, j * 128 : (j + 1) * 128],
                            ident,
                        )
                        nc.scalar.copy(vsb[:, nch * 4 + j, :], tv)

        ph1.close()

        # ---------------- phase 2: attention ----------------
        ph2 = ExitStack()
        pt_pool = ph2.enter_context(tc.tile_pool(name="pt", bufs=24))
        acc_pool = ph2.enter_context(tc.tile_pool(name="acc", bufs=10))
        cs_pool = ph2.enter_context(tc.tile_pool(name="cs", bufs=2))
        ot_pool = ph2.enter_context(tc.tile_pool(name="ot", bufs=3))
        ob_pool = ph2.enter_context(tc.tile_pool(name="ob", bufs=3))
        rc_pool = ph2.enter_context(tc.tile_pool(name="rc", bufs=4))
        ps_st = ph2.enter_context(tc.tile_pool(name="ps_st", bufs=CFG["st_bufs"], space="PSUM"))
        ps_pv = ph2.enter_context(tc.tile_pool(name="ps_pv", bufs=1, space="PSUM"))
        ps_sm = ph2.enter_context(tc.tile_pool(name="ps_sm", bufs=1, space="PSUM"))

        for g in range(NG) if "attn" not in skip else []:
            q_sl = slice(g * QG, (g + 1) * QG)
            # S^T chunks -> exp -> P^T (fp16), span by span
            pt_spans = []
            SP = spans()
            for c0, w in SP:
                stp = ps_st.tile([128, 512 * CFG["span"]], F32)
                for j in (range(w) if "st1" not in skip else [0]):
                    kc = c0 + j
                    nc.tensor.matmul(
                        stp[:, j * 512 : (j + 1) * 512],
                        lhsT=kd[:, kc * 128 : (kc + 1) * 128],
                        rhs=qd[:, q_sl],
                        start=True,
                        stop=True,
                    )
                pts = pt_pool.tile([128, 512 * CFG["span"]], F16, tag="pt")
                ew = 1 if "exp3" in skip else w
                nc.scalar.activation(
                    out=pts[:, : ew * 512],
                    in_=stp[:, : ew * 512],
                    func=AF.Exp,
                    scale=SCALE,
                    bias=expb,
                )
                pt_spans.append(pts)

            def ptc(kc):
                s = kc // CFG["span"]
                j = kc - SP[s][0]
                return pt_spans[s][:, j * 512 : (j + 1) * 512]

            # P^T V accumulation: out^T [dh, q]
            pv = ps_pv.tile([128, 512], F32, tag="pv")
            pv_rng = range(KC) if "pv8" not in skip else range(8)
            for kc in pv_rng:
                nc.tensor.matmul(
                    pv,
                    lhsT=vsb[:, kc, :],
                    rhs=ptc(kc),
                    start=(kc == 0),
                    stop=(kc == pv_rng[-1]),
                )

            # denominator: fp16 chunk add-tree (8 lanes x 4 chunks), then
            # 128-partition reduction via ones-matmul
            lanes = []
            if "tree" in skip:
                cs0 = cs_pool.tile([128, 512], F16, name="cs0", tag="cs")
                nc.vector.memset(cs0, 1.0)
            for l in range(8) if "tree" not in skip else []:
                acc = acc_pool.tile([128, 512], F16, tag="acc")
                nc.vector.tensor_add(acc, ptc(4 * l), ptc(4 * l + 1))
                nc.vector.tensor_add(acc, acc, ptc(4 * l + 2))
                nc.vector.tensor_add(acc, acc, ptc(4 * l + 3))
                lanes.append(acc)
            for i in range(4) if "tree" not in skip else []:
                nc.vector.tensor_add(lanes[i], lanes[i], lanes[i + 4])
            for i in range(2) if "tree" not in skip else []:
                nc.vector.tensor_add(lanes[i], lanes[i], lanes[i + 2])
            if "tree" not in skip:
                cs = cs_pool.tile([128, 512], F16)
                nc.vector.tensor_add(cs, lanes[0], lanes[1])
            else:
                cs = cs0

            # out^T -> SBUF, then per-subtile transpose + normalize
            ot = ot_pool.tile([128, 512], F32R)
            nc.scalar.copy(ot, pv)

            dn = ps_sm.tile([128, 4], F32, tag="sm")
            for st in range(4):
                nc.tensor.matmul(
                    dn[:, st : st + 1],
                    lhsT=cs[:, st * 128 : (st + 1) * 128],
                    rhs=ones16,
                    start=True,
                    stop=True,
                )
            rc = rc_pool.tile([128, 4], F32)
            nc.vector.reciprocal(rc, dn)

            tp = ps_sm.tile([128, 512], F32, tag="sm")
            for st in range(4):
                nc.tensor.transpose(
                    tp[:, st * 128 : (st + 1) * 128].bitcast(F32R),
                    ot[:, st * 128 : (st + 1) * 128],
                    ident,
                )
            ob = ob_pool.tile([128, 4, 128], F32)
            for st in range(4):
                nc.vector.tensor_scalar_mul(
                    ob[:, st, :], tp[:, st * 128 : (st + 1) * 128], rc[:, st : st + 1]
                )
            nc.sync.dma_start(
                out=y[q_sl, :].rearrange("(s p) d -> p s d", p=128), in_=ob
            )

        ph2.close()


_NC = None


def kernel(x, W, b):
    global _NC
    if _NC is None:
        _NC = build()

    x = np.asarray(x, dtype=np.float32)
    W = np.asarray(W, dtype=np.float32)
    b = np.asarray(b, dtype=np.float32)

    in_maps = []
    for c in range(NCORES):
        bb, h = divmod(c, HEADS)
        rows = np.arange(DH) * HEADS + h
        wt = np.concatenate(
            [np.ascontiguousarray(W[blk * DIM + rows, :].T) for blk in range(3)],
            axis=1,
        )  # [DIM, 3*DH]
        bs = np.stack([b[blk * DIM + rows] for blk in range(3)], axis=0)  # [3, DH]
        in_maps.append(
            {
                "xb": np.ascontiguousarray(x[bb]),
                "wt": np.ascontiguousarray(wt),
                "bqkv": np.ascontiguousarray(bs),
            }
        )

    res = run_bass_kernel_spmd(_NC, in_maps, core_ids=list(range(NCORES)))

    out = np.empty((B, N, HEADS * DH), dtype=np.float32)
    for c in range(NCORES):
        bb, h = divmod(c, HEADS)
        out[bb, :, h * DH : (h + 1) * DH] = res.results[c]["y"]
    return out
